# revision 1
# baseline (speedup 1.0000x reference)
"""LocalTrittention TRN2 kernel: 8-core batch-data-parallel Bass/Tile implementation.

Problem (B=64, S=256, HID=4096, H=16, D=256, WINDOW=64):
  q,k1,k2,v1,v2 = hs @ W*.T + b*            (5 projections, per-head split)
  s1 = q @ k1^T ; scores = (s1 @ k2^T) * 1/sqrt(D)   (per (b,h), S==D)
  scores[:, S-WINDOW:] = -inf ; probs = softmax(scores)
  out = probs @ (v1+v2)  -> [B,S,HID]

Sharding: batch (64) split across 8 cores (8 batches/core). Weights replicated.
Host prep: layout only (transpose hs shard and the 5 weight matrices so the
contraction index is partition-major); all FLOPs run on device.

Device math runs in fp32r (fp32 rounded to 11 mantissa bits, 4x faster
matmul); accumulation is fp32 in PSUM; softmax in fp32.
"""

import sys, time

sys.path.insert(0, "/opt/trn_rl_repo")

import numpy as np

import concourse.bass as bass
import concourse.tile as tile
from concourse import bacc, mybir
from concourse.masks import make_identity

B, S, HID = 64, 256, 4096
H, D = 16, 256
WINDOW = 64
SV = S - WINDOW  # valid (unmasked) score columns
SCALE = 1.0 / float(np.sqrt(D))

NCORES = 8
BPC = B // NCORES  # batches per core
T = BPC * S  # tokens per core (2048)
KC = HID // 128  # contraction chunks (32)
HALF = T // 2  # token half (1024)

F32 = mybir.dt.float32
F32R = mybir.dt.float32r
AX = mybir.AxisListType.X
EXP = mybir.ActivationFunctionType.Exp


def build_bass(reps=1):
    nc = bacc.Bacc("TRN2", target_bir_lowering=False, debug=True)

    hsT = nc.dram_tensor("hsT", [HID, T], F32, kind="ExternalInput")
    wts = {
        n: nc.dram_tensor(f"w{n}T", [HID, HID], F32, kind="ExternalInput")
        for n in ("q", "k1", "k2", "v1", "v2")
    }
    bqs = {
        n: nc.dram_tensor(f"b{n}", [HID], F32, kind="ExternalInput")
        for n in ("q", "k1", "k2", "v1", "v2")
    }
    outd = nc.dram_tensor("out", [T, HID], F32, kind="ExternalOutput")

    with tile.TileContext(nc) as tc:
        with (
            tc.tile_pool(name="const", bufs=1) as const,
            tc.tile_pool(name="dram", bufs=1, space="DRAM") as dram,
        ):
            ident = const.tile([128, 128], F32)
            make_identity(nc, ident[:])

            # per-partition bias tiles [128, 32] (o-chunk-major) for q/k1/k2
            bias_sb = {}
            for n in ("q", "k1", "k2"):
                t = const.tile([128, KC], F32, name=f"bias_{n}")
                nc.sync.dma_start(t[:], bqs[n].ap().rearrange("(m p) -> p m", p=128))
                bias_sb[n] = t
            # broadcast bias (bv1+bv2) [128, HID] for the ctx epilogue
            bv1b = const.tile([128, HID], F32)
            nc.sync.dma_start(bv1b[:], bqs["v1"].ap().partition_broadcast(128))
            bv2b = const.tile([128, HID], F32)
            nc.sync.dma_start(bv2b[:], bqs["v2"].ap().partition_broadcast(128))
            biasb = const.tile([128, HID], F32)
            nc.vector.tensor_add(biasb[:], bv1b[:], bv2b[:])

            # intermediate DRAM (fp32r): qT/k1T/k2T [HID, T] d-major, v [T, HID]
            qTd = dram.tile([HID, T], F32R, name="qTd")
            k1Td = dram.tile([HID, T], F32R, name="k1Td")
            k2Td = dram.tile([HID, T], F32R, name="k2Td")
            vd = dram.tile([T, HID], F32R, name="vd")
            projd = {"q": qTd, "k1": k1Td, "k2": k2Td}

            for _rep in range(reps):
                # ---------------- Phase A: projections ----------------
                with (
                    tc.tile_pool(name="hst", bufs=1) as hstp,
                    tc.tile_pool(name="wtile", bufs=6) as wtp,
                    tc.tile_pool(name="evac", bufs=6) as evp,
                    tc.tile_pool(name="apsum", bufs=8, space="PSUM") as apsum,
                ):
                    for hf in range(2):
                        cols = slice(hf * HALF, (hf + 1) * HALF)
                        hst = hstp.tile([128, KC, HALF], F32R, tag="hst", name="hst")
                        for k in range(KC):
                            nc.gpsimd.dma_start(
                                hst[:, k, :], hsT.ap()[k * 128 : (k + 1) * 128, cols]
                            )

                        # q/k1/k2: out-chunk-stationary (W tile), hsT moving
                        for n in ("q", "k1", "k2"):
                            wt = wts[n]
                            for mg in range(8):
                                pss = [
                                    apsum.tile(
                                        [128, 512], F32, tag="ps", name=f"ps{i}"
                                    )
                                    for i in range(8)
                                ]
                                for k in range(KC):
                                    wtile = wtp.tile(
                                        [128, 512], F32R, tag="wt", name="wtile"
                                    )
                                    nc.gpsimd.dma_start(
                                        wtile[:],
                                        wt.ap()[
                                            k * 128 : (k + 1) * 128,
                                            mg * 512 : (mg + 1) * 512,
                                        ],
                                    )
                                    for m in range(4):
                                        for nn in range(2):
                                            nc.tensor.matmul(
                                                pss[m * 2 + nn][:],
                                                wtile[:, m * 128 : (m + 1) * 128],
                                                hst[:, k, nn * 512 : (nn + 1) * 512],
                                                start=(k == 0),
                                                stop=(k == KC - 1),
                                            )
                                for m in range(4):
                                    for nn in range(2):
                                        ev = evp.tile(
                                            [128, 512], F32R, tag="ev", name="ev"
                                        )
                                        nc.vector.tensor_scalar_add(
                                            ev[:],
                                            pss[m * 2 + nn][:],
                                            bias_sb[n][:, mg * 4 + m : mg * 4 + m + 1],
                                        )
                                        nc.sync.dma_start(
                                            projd[n][
                                                mg * 512 + m * 128 : mg * 512 + (m + 1) * 128,
                                                hf * HALF + nn * 512 : hf * HALF + (nn + 1) * 512,
                                            ],
                                            ev[:],
                                        )

                        # v = hs@(wv1.T) + hs@(wv2.T): hsT stationary, W moving
                        for ng in range(8):
                            pss = [
                                apsum.tile([128, 512], F32, tag="ps", name=f"vps{i}")
                                for i in range(8)
                            ]
                            for k in range(KC):
                                wv_tiles = []
                                for wi, n in enumerate(("v1", "v2")):
                                    wtile = wtp.tile(
                                        [128, 512], F32R, tag="wt", name="wvtile"
                                    )
                                    nc.gpsimd.dma_start(
                                        wtile[:],
                                        wts[n].ap()[
                                            k * 128 : (k + 1) * 128,
                                            ng * 512 : (ng + 1) * 512,
                                        ],
                                    )
                                    wv_tiles.append(wtile)
                                for wi in range(2):
                                    for m in range(8):
                                        nc.tensor.matmul(
                                            pss[m][:],
                                            hst[:, k, m * 128 : (m + 1) * 128],
                                            wv_tiles[wi][:],
                                            start=(k == 0 and wi == 0),
                                            stop=(k == KC - 1 and wi == 1),
                                        )
                            for m in range(8):
                                ev = evp.tile([128, 512], F32R, tag="ev", name="vev")
                                nc.vector.tensor_copy(ev[:], pss[m][:])
                                nc.sync.dma_start(
                                    vd[
                                        hf * HALF + m * 128 : hf * HALF + (m + 1) * 128,
                                        ng * 512 : (ng + 1) * 512,
                                    ],
                                    ev[:],
                                )

                # ---------------- Phase B: attention ----------------
                with (
                    tc.tile_pool(name="bio", bufs=3) as bio,
                    tc.tile_pool(name="bwork", bufs=2) as bw,
                    tc.tile_pool(name="bps", bufs=2, space="PSUM") as bps,
                ):
                    for b in range(BPC):
                        for h in range(H):
                            rows = slice(h * S, (h + 1) * S)
                            colsb = slice(b * S, (b + 1) * S)
                            qt = bio.tile([128, 2, S], F32R, tag="qt", name="qt")
                            k1 = bio.tile([128, 2, S], F32R, tag="k1", name="k1")
                            k2 = bio.tile([128, 2, S], F32R, tag="k2", name="k2")
                            vt = bio.tile([128, 2, S], F32R, tag="vt", name="vt")
                            for t_, d_ in ((qt, qTd), (k1, k1Td), (k2, k2Td)):
                                nc.sync.dma_start(
                                    t_[:],
                                    d_[rows, colsb].rearrange("(c p) s -> p c s", p=128),
                                )
                            nc.sync.dma_start(
                                vt[:],
                                vd[colsb, rows].rearrange("(c p) s -> p c s", p=128),
                            )

                            # s1T[m,q] = sum_d k1T[d,m] qT[d,q], scaled
                            s1r = bw.tile([128, 2, S], F32R, tag="s1r", name="s1r")
                            for m in range(2):
                                ps = bps.tile([128, S], F32, tag="s1ps", name="s1ps")
                                for d_ in range(2):
                                    nc.tensor.matmul(
                                        ps[:],
                                        k1[:, d_, bass.ts(m, 128)],
                                        qt[:, d_, :],
                                        start=(d_ == 0),
                                        stop=(d_ == 1),
                                    )
                                nc.vector.tensor_scalar_mul(s1r[:, m, :], ps[:], SCALE)

                            # scores[q,j] (full N=S), softmax over j<SV
                            probs = bw.tile([128, 2, SV], F32, tag="probs", name="probs")
                            recip = bw.tile([128, 2], F32, tag="recip", name="recip")
                            for q in range(2):
                                ps = bps.tile([128, S], F32, tag="scps", name="scps")
                                for m in range(2):
                                    nc.tensor.matmul(
                                        ps[:],
                                        s1r[:, m, bass.ts(q, 128)],
                                        k2[:, m, :],
                                        start=(m == 0),
                                        stop=(m == 1),
                                    )
                                negmax = bw.tile([128, 1], F32, tag="ngm", name="ngm")
                                nc.vector.reduce_max(
                                    negmax[:], ps[:, :SV], axis=AX, negate=True
                                )
                                sumexp = bw.tile([128, 1], F32, tag="sme", name="sme")
                                nc.scalar.activation(
                                    probs[:, q, :],
                                    ps[:, :SV],
                                    EXP,
                                    bias=negmax[:],
                                    scale=1.0,
                                    accum_out=sumexp[:],
                                )
                                nc.vector.reciprocal(recip[:, q : q + 1], sumexp[:])

                            # transpose probs (valid cols only) -> fp32r
                            ptr = bw.tile([128, 2, S], F32R, tag="ptr", name="ptr")
                            for q in range(2):
                                pst = bps.tile([128, S], F32, tag="pst", name="pst")
                                nc.tensor.transpose(
                                    pst[:, bass.ts(0, 128)], probs[:, q, :128], ident[:]
                                )
                                nc.tensor.transpose(
                                    pst[:64, bass.ds(128, 128)],
                                    probs[:, q, 128:SV],
                                    ident[:],
                                )
                                nc.vector.tensor_copy(ptr[:, q, :], pst[:])

                            # ctx[q,d] = sum_{j<SV} probsT[j,q] v[j,d]; normalize+bias
                            ctxs = bw.tile([128, 2, S], F32, tag="ctxs", name="ctxs")
                            for q in range(2):
                                ps = bps.tile([128, S], F32, tag="ctxps", name="ctxps")
                                nc.tensor.matmul(
                                    ps[:],
                                    ptr[:, q, :128],
                                    vt[:, 0, :],
                                    start=True,
                                    stop=False,
                                )
                                nc.tensor.matmul(
                                    ps[:],
                                    ptr[:64, q, 128:256],
                                    vt[:64, 1, :],
                                    start=False,
                                    stop=True,
                                )
                                nc.vector.tensor_scalar_mul(
                                    ctxs[:, q, :], ps[:], recip[:, q : q + 1]
                                )
                                nc.vector.tensor_add(
                                    ctxs[:, q, :], ctxs[:, q, :], biasb[:, rows]
                                )

                            nc.sync.dma_start(
                                outd.ap()[colsb, rows].rearrange(
                                    "(c p) s -> p c s", p=128
                                ),
                                ctxs[:],
                            )
    nc.compile()
    return nc


# ---------------------------------------------------------------------------
# host-side runner (mirrors bass2jax.run_bass_via_pjrt with device-resident
# inputs; weights replicated across cores rather than concatenated)
# ---------------------------------------------------------------------------

_CACHE = {}


def _run(nc, in_maps, n_cores, replicated=(), time_reps=0):
    import jax
    from jax.sharding import Mesh, PartitionSpec, NamedSharding
    from jax.experimental.shard_map import shard_map
    from concourse.bass2jax import (
        install_neuronx_cc_hook,
        _bass_exec_p,
        partition_id_tensor,
    )

    install_neuronx_cc_hook()

    if nc.dbg_addr is not None:
        assert not nc.dbg_callbacks
        in_maps = [
            {**m, nc.dbg_addr.name: np.zeros((1, 2), np.uint32)} for m in in_maps
        ]

    partition_name = nc.partition_id_tensor.name if nc.partition_id_tensor else None

    in_names, out_names, out_avals, zero_outs = [], [], [], []
    for alloc in nc.m.functions[0].allocations:
        if not isinstance(alloc, mybir.MemoryLocationSet):
            continue
        name = alloc.memorylocations[0].name
        if alloc.kind == "ExternalInput":
            if name != partition_name:
                in_names.append(name)
        elif alloc.kind == "ExternalOutput":
            out_names.append(name)
            shape = tuple(alloc.tensor_shape)
            dtype = mybir.dt.np(alloc.dtype)
            out_avals.append(jax.core.ShapedArray(shape, dtype))
            zero_outs.append(np.zeros(shape, dtype))
    n_params = len(in_names)
    n_outs = len(out_avals)
    param_names = list(in_names)
    in_names = in_names + out_names
    if partition_name is not None:
        in_names.append(partition_name)

    donate = tuple(range(n_params, n_params + n_outs))

    def _body(*args):
        operands = list(args)
        if partition_name is not None:
            operands.append(partition_id_tensor())
        outs = _bass_exec_p.bind(
            *operands,
            out_avals=tuple(out_avals),
            in_names=tuple(in_names),
            out_names=tuple(out_names),
            lowering_input_output_aliases=(),
            sim_require_finite=True,
            sim_require_nnan=True,
            nc=nc,
        )
        return tuple(outs)

    devices = jax.devices()[:n_cores]
    mesh = Mesh(np.asarray(devices), ("core",))
    rep = set(replicated)
    in_specs = tuple(
        PartitionSpec() if nm in rep else PartitionSpec("core")
        for nm in param_names
    ) + (PartitionSpec("core"),) * n_outs
    out_specs = (PartitionSpec("core"),) * len(out_names)
    sharded = jax.jit(
        shard_map(
            _body, mesh=mesh, in_specs=in_specs, out_specs=out_specs, check_rep=False
        ),
        donate_argnums=donate,
        keep_unused=True,
    )

    shard_sh = NamedSharding(mesh, PartitionSpec("core"))
    rep_sh = NamedSharding(mesh, PartitionSpec())
    concat_in = []
    for i, nm in enumerate(param_names):
        if nm in rep:
            concat_in.append(jax.device_put(np.asarray(in_maps[0][nm]), rep_sh))
        else:
            concat_in.append(
                jax.device_put(
                    np.concatenate(
                        [np.asarray(in_maps[c][nm]) for c in range(n_cores)], axis=0
                    ),
                    shard_sh,
                )
            )
    jax.block_until_ready(concat_in)

    def fresh_zeros():
        zs = [
            jax.device_put(np.zeros((n_cores * z.shape[0], *z.shape[1:]), z.dtype), shard_sh)
            for z in zero_outs
        ]
        jax.block_until_ready(zs)
        return zs

    t0 = time.perf_counter()
    out_arrs = jax.block_until_ready(sharded(*concat_in, *fresh_zeros()))
    first_call_s = time.perf_counter() - t0
    results = [
        {
            name: np.asarray(out_arrs[i]).reshape(n_cores, *out_avals[i].shape)[c]
            for i, name in enumerate(out_names)
        }
        for c in range(n_cores)
    ]

    # non-donating variant for timing bursts: zeros stay device-resident and
    # are reused across calls (the kernel writes every output element)
    sharded_nd = jax.jit(
        shard_map(
            _body, mesh=mesh, in_specs=in_specs, out_specs=out_specs, check_rep=False
        ),
        keep_unused=True,
    )
    zs_resident = fresh_zeros()

    def timed_burst(m):
        """Enqueue m executions back-to-back, fetch a few bytes of the last
        one's output. Device serializes the execs, so wall ~= dispatch
        overhead + m * exec_time once m*exec exceeds the RPC window."""
        t0 = time.perf_counter()
        outs = None
        for _ in range(m):
            outs = sharded_nd(*concat_in, *zs_resident)
        for o in outs:
            np.asarray(jax.device_get(o.addressable_shards[0].data[0:1, 0:8]))
        return time.perf_counter() - t0

    times = [timed_burst(1) for _ in range(time_reps)]

    return results, times, first_call_s, timed_burst


def kernel(
    hidden_states,
    wq,
    bq,
    wk1,
    bk1,
    wk2,
    bk2,
    wv1,
    bv1,
    wv2,
    bv2,
    _time_reps=0,
    _reps=1,
):
    hs = np.asarray(hidden_states, dtype=np.float32)
    weights = {
        "q": np.asarray(wq, np.float32),
        "k1": np.asarray(wk1, np.float32),
        "k2": np.asarray(wk2, np.float32),
        "v1": np.asarray(wv1, np.float32),
        "v2": np.asarray(wv2, np.float32),
    }
    biases = {
        "q": np.asarray(bq, np.float32),
        "k1": np.asarray(bk1, np.float32),
        "k2": np.asarray(bk2, np.float32),
        "v1": np.asarray(bv1, np.float32),
        "v2": np.asarray(bv2, np.float32),
    }

    if ("nc", _reps) not in _CACHE:
        _CACHE[("nc", _reps)] = build_bass(_reps)
    nc = _CACHE[("nc", _reps)]

    # host prep: layout only (transposes), no arithmetic
    wT = {n: np.ascontiguousarray(w.T) for n, w in weights.items()}
    in_maps = []
    for c in range(NCORES):
        shard = hs[c * BPC : (c + 1) * BPC].reshape(T, HID)
        m = {"hsT": np.ascontiguousarray(shard.T)}
        for n in ("q", "k1", "k2", "v1", "v2"):
            m[f"w{n}T"] = wT[n]
            m[f"b{n}"] = biases[n]
        in_maps.append(m)

    replicated = [f"w{n}T" for n in weights] + [f"b{n}" for n in biases]
    results, times, first_s, burst = _run(
        nc, in_maps, NCORES, replicated=replicated, time_reps=_time_reps
    )
    kernel._last_times = times
    kernel._first_call_s = first_s
    kernel._burst = burst

    out = np.empty((B, S, HID), np.float32)
    for c in range(NCORES):
        out[c * BPC : (c + 1) * BPC] = results[c]["out"].reshape(BPC, S, HID)
    return out



# revision 26
# speedup vs baseline: 15.5048x; 15.5048x over previous
"""LocalTrittention TRN2 kernel: 8-core batch-data-parallel Bass/Tile implementation.

Problem (B=64, S=256, HID=4096, H=16, D=256, WINDOW=64):
  q,k1,k2,v1,v2 = hs @ W*.T + b*            (5 projections, per-head split)
  s1 = q @ k1^T ; scores = (s1 @ k2^T) * 1/sqrt(D)   (per (b,h), S==D)
  scores[:, S-WINDOW:] = -inf ; probs = softmax(scores)
  out = probs @ (v1+v2)  -> [B,S,HID]

Sharding: batch (64) split across 8 cores (8 batches/core). Weights replicated.
Host prep: layout only (transpose hs shard and the 5 weight matrices so the
contraction index is partition-major); all FLOPs run on device.

Device structure (v2):
  - V is fused on device: wv = wv1+wv2 (Pool adds per tile), so only 4 GEMMs.
  - k2/v are only needed for the first 192 of every 256 tokens (the last
    WINDOW=64 score columns are masked out) -> their GEMMs stream strided
    2-batch x 192-token access patterns (25% fewer PE cycles).
  - Projections are emitted in 16 groups of 512 out-dims (2 heads); the
    attention for a group is interleaved one group behind, so its DVE/Act/
    DMA work hides under the next group's PE-bound GEMMs.
  - All math in fp32r (fp32 rounded to 11 mantissa bits, 4x matmul rate);
    accumulation fp32 in PSUM; softmax fp32.
"""

import sys, time

sys.path.insert(0, "/opt/trn_rl_repo")

import numpy as np

import concourse.bass as bass
import concourse.tile as tile
from concourse import bacc, mybir
from concourse.masks import make_identity

B, S, HID = 64, 256, 4096
H, D = 16, 256
WINDOW = 64
SV = S - WINDOW  # valid (unmasked) score columns = 192
SCALE = 1.0 / float(np.sqrt(D))

NCORES = 8
BPC = B // NCORES  # batches per core (8)
T = BPC * S  # tokens per core (2048)
KC = HID // 128  # contraction chunks (32)
HALF = T // 2  # token half (1024)
NB = BPC // 2  # batches per half (4)
NG = 8  # out-dim groups of 512 (2 heads each)

F32 = mybir.dt.float32
F32R = mybir.dt.float32r
AX = mybir.AxisListType.X
EXP = mybir.ActivationFunctionType.Exp


def build_bass(reps=1):
    nc = bacc.Bacc("TRN2", target_bir_lowering=False, debug=True)

    # declared fp32r (same 4-byte layout as the fp32 host arrays) so weight
    # and hst loads are cast-free and can issue from any engine's DMA queue
    hsT = nc.dram_tensor("hsT", [HID, T], F32R, kind="ExternalInput")
    wts = {
        n: nc.dram_tensor(f"w{n}T", [HID, HID], F32R, kind="ExternalInput")
        for n in ("q", "k1", "k2", "v1", "v2")
    }
    bqs = {
        n: nc.dram_tensor(f"b{n}", [HID], F32, kind="ExternalInput")
        for n in ("q", "k1", "k2", "v1", "v2")
    }
    outd = nc.dram_tensor("out", [BPC, H, S, D], F32, kind="ExternalOutput")

    with tile.TileContext(nc) as tc:
        with (
            tc.tile_pool(name="const", bufs=1) as const,
            tc.tile_pool(name="dram", bufs=1, space="DRAM") as dram,
            tc.tile_pool(name="hstp", bufs=1) as hstp,
            tc.tile_pool(name="wtp", bufs=8) as wtp,
            tc.tile_pool(name="evp", bufs=4) as evp,
            tc.tile_pool(name="psp", bufs=8, space="PSUM") as psp,
            tc.tile_pool(name="bio", bufs=3) as bio,
            tc.tile_pool(name="bwp", bufs=2) as bwp,
        ):
            ident = const.tile([128, 128], F32)
            make_identity(nc, ident[:])

            # per-partition bias tiles [128, 32] (out-chunk-major) for q/k1/k2
            bias_sb = {}
            for n in ("q", "k1", "k2"):
                t = const.tile([128, KC], F32, name=f"bias_{n}")
                nc.sync.dma_start(t[:], bqs[n].ap().rearrange("(m p) -> p m", p=128))
                bias_sb[n] = t
            # broadcast bias (bv1+bv2) [128, HID] for the ctx epilogue;
            # bv2 is added chunk-wise through evac-pool tiles (no 16KB temp)
            biasb = const.tile([128, HID], F32)
            nc.sync.dma_start(biasb[:], bqs["v1"].ap().partition_broadcast(128))
            for c in range(8):
                bt = evp.tile([128, 512], F32R, tag="ev", name="bvtmp")
                nc.gpsimd.dma_start(
                    bt[:], bqs["v2"].ap()[c * 512 : (c + 1) * 512].partition_broadcast(128)
                )
                nc.gpsimd.tensor_add(
                    biasb[:, c * 512 : (c + 1) * 512],
                    biasb[:, c * 512 : (c + 1) * 512],
                    bt[:],
                )

            # intermediate DRAM (fp32r), one tile per out-dim group g,
            # blocked so both the evac stores and the attention loads are
            # contiguous (1-2 segment) DMAs:
            #   qTd/k1Td: [b, hloc, c, 128, 256]  (c = d-chunk)
            #   k2Td:     [b, hloc, c, 128, SV]
            #   vd:       [b, hloc, SV, 256]      (token-major v block)
            qTd = [
                dram.tile([BPC, 2, 2, 128, 256], F32R, name=f"qTd{g}")
                for g in range(NG)
            ]
            k1Td = [
                dram.tile([BPC, 2, 2, 128, 256], F32R, name=f"k1Td{g}")
                for g in range(NG)
            ]
            k2Td = [
                dram.tile([BPC, 2, 2, 128, SV], F32R, name=f"k2Td{g}")
                for g in range(NG)
            ]
            vd = [
                dram.tile([BPC, 2, SV, 256], F32R, name=f"vd{g}")
                for g in range(NG)
            ]

            # per-k hst tiles so the second half's loads only WAR-wait on
            # the matching k chunk (no whole-half barrier)
            hst = [
                hstp.tile([128, HALF], F32R, tag=f"h{k}", name=f"hst{k}")
                for k in range(KC)
            ]

            def load_hst(hf):
                cols = slice(hf * HALF, (hf + 1) * HALF)
                for k in range(KC):
                    eng = nc.sync if k % 2 == 0 else nc.scalar
                    eng.dma_start(
                        hst[k][:], hsT.ap()[k * 128 : (k + 1) * 128, cols]
                    )

            def gemm_qk1(n, hf, g):
                """q/k1: out-chunk-stationary, full 1024-token moving."""
                dst = qTd[g] if n == "q" else k1Td[g]
                pss = [
                    psp.tile([128, 512], F32, tag="ps", name=f"ps_{n}{i}")
                    for i in range(8)
                ]
                for k in range(KC):
                    wt = wtp.tile([128, 512], F32R, tag="wt", name="wt")
                    nc.gpsimd.dma_start(
                        wt[:],
                        wts[n].ap()[k * 128 : (k + 1) * 128, g * 512 : (g + 1) * 512],
                    )
                    for m in range(4):
                        for nn in range(2):
                            nc.tensor.matmul(
                                pss[m * 2 + nn][:],
                                wt[:, m * 128 : (m + 1) * 128],
                                hst[k][:, nn * 512 : (nn + 1) * 512],
                                start=(k == 0),
                                stop=(k == KC - 1),
                            )
                for m in range(4):
                    for nn in range(2):
                        ev = evp.tile([128, 512], F32R, tag="ev", name="ev")
                        nc.vector.tensor_scalar_add(
                            ev[:],
                            pss[m * 2 + nn][:],
                            bias_sb[n][:, g * 4 + m : g * 4 + m + 1],
                        )
                        b0 = hf * NB + 2 * nn
                        nc.sync.dma_start(
                            dst[b0 : b0 + 2, m // 2, m % 2].rearrange(
                                "b p s -> p b s"
                            ),
                            ev[:],
                        )

            def gemm_k2(hf, g):
                """k2: only first SV=192 tokens per batch; moving = 2-batch
                strided [2,192] patterns (384 wide, full PE rate)."""
                pss = [
                    psp.tile([128, 512], F32, tag="ps", name=f"ps_k2{i}")
                    for i in range(8)
                ]
                for k in range(KC):
                    wt = wtp.tile([128, 512], F32R, tag="wt", name="wt")
                    nc.gpsimd.dma_start(
                        wt[:],
                        wts["k2"].ap()[
                            k * 128 : (k + 1) * 128, g * 512 : (g + 1) * 512
                        ],
                    )
                    mv = hst[k][:].rearrange("p (b s) -> p b s", b=NB)
                    for m in range(4):
                        for pr in range(2):
                            nc.tensor.matmul(
                                pss[m * 2 + pr][:, :384],
                                wt[:, m * 128 : (m + 1) * 128],
                                mv[:, 2 * pr : 2 * pr + 2, :SV],
                                start=(k == 0),
                                stop=(k == KC - 1),
                            )
                for m in range(4):
                    for pr in range(2):
                        ev = evp.tile([128, 512], F32R, tag="ev", name="ev")
                        nc.vector.tensor_scalar_add(
                            ev[:, :384],
                            pss[m * 2 + pr][:, :384],
                            bias_sb["k2"][:, g * 4 + m : g * 4 + m + 1],
                        )
                        b0 = hf * NB + 2 * pr
                        nc.sync.dma_start(
                            k2Td[g][b0 : b0 + 2, m // 2, m % 2].rearrange(
                                "b p s -> p b s"
                            ),
                            ev[:, :384],
                        )

            def gemm_v(hf, g):
                """v = hs@(wv1+wv2).T; weights fused just-in-time in SBUF
                (w1/w2 stream on gpsimd, Pool adds into the wt-tag tile 8
                chunks ahead of consumption). Token-stationary; only the
                first 192 tokens of each batch (4 full 128-chunks + 2
                merged 2x64-chunks)."""
                pss = [
                    psp.tile([128, 512], F32, tag="ps", name=f"ps_v{i}")
                    for i in range(8)
                ]
                for k in range(KC):
                    w1 = evp.tile([128, 512], F32R, tag="ev", name="wv1t")
                    nc.scalar.dma_start(
                        w1[:],
                        wts["v1"].ap()[
                            k * 128 : (k + 1) * 128, g * 512 : (g + 1) * 512
                        ],
                    )
                    w2 = evp.tile([128, 512], F32R, tag="ev", name="wv2t")
                    nc.sync.dma_start(
                        w2[:],
                        wts["v2"].ap()[
                            k * 128 : (k + 1) * 128, g * 512 : (g + 1) * 512
                        ],
                    )
                    wvt = wtp.tile([128, 512], F32R, tag="wt", name="wvt")
                    nc.gpsimd.tensor_add(wvt[:], w1[:], w2[:])
                    hb = hst[k][:].rearrange("p (b s) -> p b s", b=NB)
                    for lb in range(4):
                        nc.tensor.matmul(
                            pss[lb][:],
                            hst[k][:, lb * 256 : lb * 256 + 128],
                            wvt[:],
                            start=(k == 0),
                            stop=(k == KC - 1),
                        )
                    for lb in range(4):
                        nc.tensor.matmul(
                            pss[4 + lb][:64, :],
                            hst[k][:, lb * 256 + 128 : lb * 256 + SV],
                            wvt[:],
                            start=(k == 0),
                            stop=(k == KC - 1),
                        )
                for lb in range(4):
                    ev = evp.tile([128, 512], F32R, tag="ev", name="ev")
                    nc.scalar.activation(ev[:], pss[lb][:], mybir.ActivationFunctionType.Copy)
                    b0 = hf * NB + lb
                    nc.sync.dma_start(
                        vd[g][b0, :, 0:128, :].rearrange("h p s -> p h s"),
                        ev[:],
                    )
                for lb in range(4):
                    ev = evp.tile([128, 512], F32R, tag="ev", name="ev")
                    nc.scalar.activation(
                        ev[:64, :], pss[4 + lb][:64, :],
                        mybir.ActivationFunctionType.Copy,
                    )
                    b0 = hf * NB + lb
                    nc.sync.dma_start(
                        vd[g][b0, :, 128:SV, :].rearrange("h p s -> p h s"),
                        ev[:64, :],
                    )

            def attn_front(hf, g, b, hloc):
                """loads + s1 + scores + softmax for (batch b, head 2g+hloc).
                Returns state for attn_back; the PE transpose/PV half is
                emitted one bh later so softmax latency never stalls PE."""
                roff = hloc * 256
                hid0 = g * 512 + roff  # global hid offset of this head
                qt = bio.tile([128, 2, 256], F32R, tag="qt", name="qt")
                nc.sync.dma_start(
                    qt[:], qTd[g][b, hloc].rearrange("c p s -> p c s")
                )
                k1t = bio.tile([128, 2, 256], F32R, tag="k1t", name="k1t")
                nc.sync.dma_start(
                    k1t[:], k1Td[g][b, hloc].rearrange("c p s -> p c s")
                )
                k2t = bio.tile([128, 2, SV], F32R, tag="k2t", name="k2t")
                nc.scalar.dma_start(
                    k2t[:], k2Td[g][b, hloc].rearrange("c p s -> p c s")
                )
                vt = bio.tile([128, 2, 256], F32R, tag="vt", name="vt")
                nc.scalar.dma_start(vt[:, 0, :], vd[g][b, hloc, 0:128, :])
                nc.scalar.dma_start(vt[:64, 1, :], vd[g][b, hloc, 128:SV, :])

                # s1T[m, q] = sum_d k1T[d, m] qT[d, q]   (2 m-chunks in 1 bank)
                s1ps = psp.tile([128, 512], F32, tag="ps", name="s1ps")
                for m in range(2):
                    for d_ in range(2):
                        nc.tensor.matmul(
                            s1ps[:, m * 256 : (m + 1) * 256],
                            k1t[:, d_, m * 128 : (m + 1) * 128],
                            qt[:, d_, :],
                            start=(d_ == 0),
                            stop=(d_ == 1),
                        )
                s1r = bwp.tile([128, 512], F32R, tag="s1r", name="s1r")
                nc.vector.tensor_scalar_mul(s1r[:], s1ps[:], SCALE)

                # scores[q, j<SV] = sum_m s1[q, m] k2T[m, j]  (2 q-chunks)
                scps = psp.tile([128, 512], F32, tag="ps", name="scps")
                for q in range(2):
                    for m in range(2):
                        nc.tensor.matmul(
                            scps[:, q * 256 : q * 256 + SV],
                            s1r[:, m * 256 + q * 128 : m * 256 + (q + 1) * 128],
                            k2t[:, m, :],
                            start=(m == 0),
                            stop=(m == 1),
                        )

                probs = bwp.tile([128, 2, SV], F32, tag="probs", name="probs")
                recip = bwp.tile([128, 2], F32, tag="recip", name="recip")
                for q in range(2):
                    negmax = bwp.tile([128, 1], F32, tag="ngm", name="ngm")
                    nc.vector.reduce_max(
                        negmax[:], scps[:, q * 256 : q * 256 + SV], axis=AX, negate=True
                    )
                    sumexp = bwp.tile([128, 1], F32, tag="sme", name="sme")
                    nc.scalar.activation(
                        probs[:, q, :],
                        scps[:, q * 256 : q * 256 + SV],
                        EXP,
                        bias=negmax[:],
                        scale=1.0,
                        accum_out=sumexp[:],
                    )
                    nc.vector.reciprocal(recip[:, q : q + 1], sumexp[:])
                return (b, hid0, vt, probs, recip)

            def attn_back(st):
                """transpose + PV + epilogue for a state from attn_front."""
                b, hid0, vt, probs, recip = st
                pst = psp.tile([128, 512], F32, tag="ps", name="pst")
                for q in range(2):
                    nc.tensor.transpose(
                        pst[:, q * 256 : q * 256 + 128], probs[:, q, 0:128], ident[:]
                    )
                    nc.tensor.transpose(
                        pst[:64, q * 256 + 128 : q * 256 + 256],
                        probs[:, q, 128:SV],
                        ident[:],
                    )
                ptr = bwp.tile([128, 512], F32R, tag="ptr", name="ptr")
                for q in range(2):
                    nc.vector.tensor_copy(
                        ptr[:, q * 256 : q * 256 + 128],
                        pst[:, q * 256 : q * 256 + 128],
                    )
                    nc.vector.tensor_copy(
                        ptr[:64, q * 256 + 128 : q * 256 + 256],
                        pst[:64, q * 256 + 128 : q * 256 + 256],
                    )

                # ctx[q, d] = sum_{j<SV} probsT[j, q] v[j, d]; normalize+bias
                ctxps = psp.tile([128, 512], F32, tag="ps", name="ctxps")
                for q in range(2):
                    nc.tensor.matmul(
                        ctxps[:, q * 256 : (q + 1) * 256],
                        ptr[:, q * 256 : q * 256 + 128],
                        vt[:, 0, :],
                        start=True,
                        stop=False,
                    )
                    nc.tensor.matmul(
                        ctxps[:, q * 256 : (q + 1) * 256],
                        ptr[:64, q * 256 + 128 : q * 256 + 256],
                        vt[:64, 1, :],
                        start=False,
                        stop=True,
                    )
                ctxs = bwp.tile([128, 2, 256], F32, tag="ctxs", name="ctxs")
                for q in range(2):
                    nc.vector.tensor_scalar_mul(
                        ctxs[:, q, :],
                        ctxps[:, q * 256 : (q + 1) * 256],
                        recip[:, q : q + 1],
                    )
                    nc.vector.tensor_add(
                        ctxs[:, q, :], ctxs[:, q, :], biasb[:, hid0 : hid0 + 256]
                    )
                nc.sync.dma_start(
                    outd.ap()[b, hid0 // 256].rearrange("(c p) s -> p c s", p=128),
                    ctxs[:],
                )

            attn_pend = []

            def attn_emit(hf, g, part):
                """half the attention of group (hf, g): part 0 -> batches
                (0,1) of the half, part 1 -> (2,3); both heads each. One-deep
                software pipeline: each back is emitted under the next front
                (carrying across parts and groups)."""
                for lb in (2 * part, 2 * part + 1):
                    b = hf * NB + lb
                    for hloc in range(2):
                        st = attn_front(hf, g, b, hloc)
                        if attn_pend:
                            attn_back(attn_pend.pop(0))
                        attn_pend.append(st)

            def attn_flush():
                while attn_pend:
                    attn_back(attn_pend.pop(0))

            for _rep in range(reps):
                prev = None
                for hf in range(2):
                    load_hst(hf)
                    for g in range(NG):
                        gemm_qk1("q", hf, g)
                        if prev is not None:
                            attn_emit(prev[0], prev[1], 0)
                        gemm_qk1("k1", hf, g)
                        if prev is not None:
                            attn_emit(prev[0], prev[1], 1)
                        gemm_k2(hf, g)
                        gemm_v(hf, g)
                        prev = (hf, g)
                attn_emit(prev[0], prev[1], 0)
                attn_emit(prev[0], prev[1], 1)
                attn_flush()

    nc.compile()
    return nc


# ---------------------------------------------------------------------------
# host-side runner (mirrors bass2jax.run_bass_via_pjrt with device-resident
# inputs; weights replicated across cores rather than concatenated)
# ---------------------------------------------------------------------------

_CACHE = {}


def _run(nc, in_maps, n_cores, replicated=(), time_reps=0):
    import jax
    from jax.sharding import Mesh, PartitionSpec, NamedSharding
    from jax.experimental.shard_map import shard_map
    from concourse.bass2jax import (
        install_neuronx_cc_hook,
        _bass_exec_p,
        partition_id_tensor,
    )

    install_neuronx_cc_hook()

    if nc.dbg_addr is not None:
        assert not nc.dbg_callbacks
        in_maps = [
            {**m, nc.dbg_addr.name: np.zeros((1, 2), np.uint32)} for m in in_maps
        ]

    partition_name = nc.partition_id_tensor.name if nc.partition_id_tensor else None

    in_names, out_names, out_avals, zero_outs = [], [], [], []
    for alloc in nc.m.functions[0].allocations:
        if not isinstance(alloc, mybir.MemoryLocationSet):
            continue
        name = alloc.memorylocations[0].name
        if alloc.kind == "ExternalInput":
            if name != partition_name:
                in_names.append(name)
        elif alloc.kind == "ExternalOutput":
            out_names.append(name)
            shape = tuple(alloc.tensor_shape)
            dtype = mybir.dt.np(alloc.dtype)
            out_avals.append(jax.core.ShapedArray(shape, dtype))
            zero_outs.append(np.zeros(shape, dtype))
    n_params = len(in_names)
    n_outs = len(out_avals)
    param_names = list(in_names)
    in_names = in_names + out_names
    if partition_name is not None:
        in_names.append(partition_name)

    donate = tuple(range(n_params, n_params + n_outs))

    def _body(*args):
        operands = list(args)
        if partition_name is not None:
            operands.append(partition_id_tensor())
        outs = _bass_exec_p.bind(
            *operands,
            out_avals=tuple(out_avals),
            in_names=tuple(in_names),
            out_names=tuple(out_names),
            lowering_input_output_aliases=(),
            sim_require_finite=True,
            sim_require_nnan=True,
            nc=nc,
        )
        return tuple(outs)

    devices = jax.devices()[:n_cores]
    mesh = Mesh(np.asarray(devices), ("core",))
    rep = set(replicated)
    in_specs = tuple(
        PartitionSpec() if nm in rep else PartitionSpec("core")
        for nm in param_names
    ) + (PartitionSpec("core"),) * n_outs
    out_specs = (PartitionSpec("core"),) * len(out_names)
    sharded = jax.jit(
        shard_map(
            _body, mesh=mesh, in_specs=in_specs, out_specs=out_specs, check_rep=False
        ),
        donate_argnums=donate,
        keep_unused=True,
    )

    shard_sh = NamedSharding(mesh, PartitionSpec("core"))
    rep_sh = NamedSharding(mesh, PartitionSpec())
    concat_in = []
    for i, nm in enumerate(param_names):
        if nm in rep:
            concat_in.append(jax.device_put(np.asarray(in_maps[0][nm]), rep_sh))
        else:
            concat_in.append(
                jax.device_put(
                    np.concatenate(
                        [np.asarray(in_maps[c][nm]) for c in range(n_cores)], axis=0
                    ),
                    shard_sh,
                )
            )
    jax.block_until_ready(concat_in)

    def fresh_zeros():
        zs = [
            jax.device_put(np.zeros((n_cores * z.shape[0], *z.shape[1:]), z.dtype), shard_sh)
            for z in zero_outs
        ]
        jax.block_until_ready(zs)
        return zs

    t0 = time.perf_counter()
    out_arrs = jax.block_until_ready(sharded(*concat_in, *fresh_zeros()))
    first_call_s = time.perf_counter() - t0
    results = [
        {
            name: np.asarray(out_arrs[i]).reshape(n_cores, *out_avals[i].shape)[c]
            for i, name in enumerate(out_names)
        }
        for c in range(n_cores)
    ]

    # non-donating variant for timing bursts: zeros stay device-resident and
    # are reused across calls (the kernel writes every output element)
    sharded_nd = jax.jit(
        shard_map(
            _body, mesh=mesh, in_specs=in_specs, out_specs=out_specs, check_rep=False
        ),
        keep_unused=True,
    )
    zs_resident = fresh_zeros()

    def timed_burst(m):
        """Enqueue m executions back-to-back, fetch a few bytes of the last
        one's output. Device serializes the execs, so wall ~= dispatch
        overhead + m * exec_time once m*exec exceeds the RPC window."""
        t0 = time.perf_counter()
        outs = None
        for _ in range(m):
            outs = sharded_nd(*concat_in, *zs_resident)
        for o in outs:
            np.asarray(jax.device_get(o.addressable_shards[0].data[0:1, 0:8]))
        return time.perf_counter() - t0

    times = [timed_burst(1) for _ in range(time_reps)]

    return results, times, first_call_s, timed_burst


def kernel(
    hidden_states,
    wq,
    bq,
    wk1,
    bk1,
    wk2,
    bk2,
    wv1,
    bv1,
    wv2,
    bv2,
    _time_reps=0,
    _reps=1,
):
    hs = np.asarray(hidden_states, dtype=np.float32)
    weights = {
        "q": np.asarray(wq, np.float32),
        "k1": np.asarray(wk1, np.float32),
        "k2": np.asarray(wk2, np.float32),
        "v1": np.asarray(wv1, np.float32),
        "v2": np.asarray(wv2, np.float32),
    }
    biases = {
        "q": np.asarray(bq, np.float32),
        "k1": np.asarray(bk1, np.float32),
        "k2": np.asarray(bk2, np.float32),
        "v1": np.asarray(bv1, np.float32),
        "v2": np.asarray(bv2, np.float32),
    }

    if ("nc", _reps) not in _CACHE:
        _CACHE[("nc", _reps)] = build_bass(_reps)
    nc = _CACHE[("nc", _reps)]

    # host prep: layout only (transposes), no arithmetic
    wT = {n: np.ascontiguousarray(w.T) for n, w in weights.items()}
    in_maps = []
    for c in range(NCORES):
        shard = hs[c * BPC : (c + 1) * BPC].reshape(T, HID)
        m = {"hsT": np.ascontiguousarray(shard.T)}
        for n in ("q", "k1", "k2", "v1", "v2"):
            m[f"w{n}T"] = wT[n]
            m[f"b{n}"] = biases[n]
        in_maps.append(m)

    replicated = [f"w{n}T" for n in weights] + [f"b{n}" for n in biases]
    results, times, first_s, burst = _run(
        nc, in_maps, NCORES, replicated=replicated, time_reps=_time_reps
    )
    kernel._last_times = times
    kernel._first_call_s = first_s
    kernel._burst = burst

    out = np.empty((B, S, HID), np.float32)
    for c in range(NCORES):
        blk = results[c]["out"]  # [BPC, H, S, D]
        out[c * BPC : (c + 1) * BPC] = (
            blk.transpose(0, 2, 1, 3).reshape(BPC, S, HID)
        )
    return out
